# revision 1
# baseline (speedup 1.0000x reference)
"""Trainium2 Bass kernel for the CorefLinker loss (data-parallel over batch).

Math: with E = exp(scores) (safe unshifted — scores ~ N(0,1)), per row m of
batch b the reference loss reduces to

    loss[b,m] = log(sum_unmasked E) - log(sum_target E)        for m < lengths[b]

where the unmasked set is {c < candidate_lengths[b,m]} ∪ the full coref block,
and the target set is {c < cl, link_targets=1} ∪ {j : same cluster, j != m}
∪ {m itself iff no other target exists}.  Masked/non-target terms in the
reference contribute exactly 0 in f32 (exp underflow), so this is exact.

Each of the 8 cores processes one batch element; the host sums 8 partials.
"""

from contextlib import ExitStack

import numpy as np

import concourse.bass as bass
import concourse.tile as tile
from concourse import bacc, mybir
from concourse.bass_utils import run_bass_kernel_spmd

P = 128          # SBUF partitions
C = 32           # linker candidates
M = 1024         # spans per batch
T = M // P       # row tiles per core
B = 8            # batch == number of cores
F32 = mybir.dt.float32

_CACHE = {}


def _build_program():
    nc = bacc.Bacc("TRN2", target_bir_lowering=False, debug=False)

    scores = nc.dram_tensor("scores", [M, C + M], F32, kind="ExternalInput").ap()
    linkf = nc.dram_tensor("linkf", [M, C], F32, kind="ExternalInput").ap()
    # aux columns (all f32, [P, col] layout so aux[p, k*T + t] is row m=t*P+p):
    #   [0:T) cidA (row-side cluster id, -1 if invalid)
    #   [T:2T) candidate_lengths
    #   [2T:3T) valid (cid >= 0)
    #   [3T:4T) in_range (m < lengths[b])
    #   [4T:4T+C) iota 0..31 (same in every partition)
    aux = nc.dram_tensor("aux", [P, 4 * T + C], F32, kind="ExternalInput").ap()
    # column-side cluster ids, -2 if invalid (so invalid never matches)
    cidb = nc.dram_tensor("cidb", [M], F32, kind="ExternalInput").ap()
    out = nc.dram_tensor("out", [1, 1], F32, kind="ExternalOutput").ap()

    eq = mybir.AluOpType.is_equal
    lt = mybir.AluOpType.is_lt
    mul = mybir.AluOpType.mult
    add = mybir.AluOpType.add
    sub = mybir.AluOpType.subtract
    EXP = mybir.ActivationFunctionType.Exp
    LN = mybir.ActivationFunctionType.Ln

    with TileCtx(nc) as (tc, ctx):
        singles = ctx.enter_context(tc.tile_pool(name="singles", bufs=1))
        sc_pool = ctx.enter_context(tc.tile_pool(name="sc", bufs=3))
        e_pool = ctx.enter_context(tc.tile_pool(name="e", bufs=2))
        scr_pool = ctx.enter_context(tc.tile_pool(name="scr", bufs=2))
        lk_pool = ctx.enter_context(tc.tile_pool(name="lk", bufs=3))
        sm_pool = ctx.enter_context(tc.tile_pool(name="sm", bufs=4))
        ps_pool = ctx.enter_context(
            tc.tile_pool(name="ps", bufs=1, space=bass.MemorySpace.PSUM)
        )

        aux_t = singles.tile([P, 4 * T + C], F32)
        nc.sync.dma_start(aux_t[:], aux)
        cidA = aux_t[:, 0 * T : 1 * T]
        clen = aux_t[:, 1 * T : 2 * T]
        valid = aux_t[:, 2 * T : 3 * T]
        inr = aux_t[:, 3 * T : 4 * T]
        iota32 = aux_t[:, 4 * T : 4 * T + C]

        # broadcast column-side cids to all partitions
        cidb_b = singles.tile([P, M], F32)
        nc.sync.dma_start(
            cidb_b[:], bass.AP(tensor=cidb.tensor, offset=0, ap=[[0, P], [1, M]])
        )

        # diagonal scores s[m, C+m]  ->  sdiag[p, t], m = t*P + p
        sdiag = singles.tile([P, T], F32)
        nc.sync.dma_start(
            sdiag[:],
            bass.AP(
                tensor=scores.tensor,
                offset=C,
                ap=[[(C + M + 1), P], [(C + M + 1) * P, T]],
            ),
        )
        e_diag = singles.tile([P, T], F32)
        nc.scalar.activation(e_diag[:], sdiag[:], EXP)

        # per-row stats, one column per row tile
        den_cor = singles.tile([P, T], F32)
        corefE = singles.tile([P, T], F32)
        tgt_lnk = singles.tile([P, T], F32)
        den_lnk = singles.tile([P, T], F32)

        for t in range(T):
            rows = slice(t * P, (t + 1) * P)
            s_t = sc_pool.tile([P, C + M], F32)
            nc.sync.dma_start(s_t[:], scores[rows, :])
            lk_t = lk_pool.tile([P, C], F32)
            nc.sync.dma_start(lk_t[:], linkf[rows, :])

            e_t = e_pool.tile([P, C + M], F32)
            nc.scalar.activation(
                e_t[:, C:], s_t[:, C:], EXP, accum_out=den_cor[:, t : t + 1]
            )
            nc.scalar.activation(e_t[:, 0:C], s_t[:, 0:C], EXP)

            # sum_j (cid_j == cid_i) * E[i, C+j]   (includes self when valid)
            scr = scr_pool.tile([P, M], F32)
            nc.vector.scalar_tensor_tensor(
                scr[:],
                cidb_b[:],
                cidA[:, t : t + 1],
                e_t[:, C:],
                op0=eq,
                op1=mul,
                accum_out=corefE[:, t : t + 1],
            )
            # linker: targets and denominator, masked by c < candidate_len
            lke = sm_pool.tile([P, C], F32)
            nc.vector.tensor_tensor(lke[:], e_t[:, 0:C], lk_t[:], op=mul)
            sm1 = sm_pool.tile([P, C], F32)
            nc.vector.scalar_tensor_tensor(
                sm1[:],
                iota32,
                clen[:, t : t + 1],
                lke[:],
                op0=lt,
                op1=mul,
                accum_out=tgt_lnk[:, t : t + 1],
            )
            sm2 = sm_pool.tile([P, C], F32)
            nc.vector.scalar_tensor_tensor(
                sm2[:],
                iota32,
                clen[:, t : t + 1],
                e_t[:, 0:C],
                op0=lt,
                op1=mul,
                accum_out=den_lnk[:, t : t + 1],
            )

        # row-wise epilogue on [P, T] tiles
        denom = singles.tile([P, T], F32)
        nc.vector.tensor_tensor(denom[:], den_cor[:], den_lnk[:], op=add)
        vE = singles.tile([P, T], F32)
        nc.vector.tensor_tensor(vE[:], valid, e_diag[:], op=mul)
        numer = singles.tile([P, T], F32)
        nc.vector.tensor_tensor(numer[:], corefE[:], vE[:], op=sub)
        nc.vector.tensor_tensor(numer[:], numer[:], tgt_lnk[:], op=add)
        sing = singles.tile([P, T], F32)
        nc.vector.tensor_scalar(sing[:], numer[:], 0.0, None, op0=eq)
        nc.vector.tensor_tensor(sing[:], sing[:], e_diag[:], op=mul)
        nc.vector.tensor_tensor(numer[:], numer[:], sing[:], op=add)

        lden = singles.tile([P, T], F32)
        nc.scalar.activation(lden[:], denom[:], LN)
        lnum = singles.tile([P, T], F32)
        nc.scalar.activation(lnum[:], numer[:], LN)
        loss = singles.tile([P, T], F32)
        nc.vector.tensor_tensor(loss[:], lden[:], lnum[:], op=sub)
        nc.vector.tensor_tensor(loss[:], loss[:], inr, op=mul)

        rowsum = singles.tile([P, 1], F32)
        nc.vector.tensor_reduce(
            rowsum[:], loss[:], axis=mybir.AxisListType.X, op=add
        )
        ones = singles.tile([P, 1], F32)
        nc.vector.memset(ones[:], 1.0)
        tot_ps = ps_pool.tile([1, 1], F32)
        nc.tensor.matmul(tot_ps[:], rowsum[:], ones[:], start=True, stop=True)
        tot = singles.tile([1, 1], F32)
        nc.vector.tensor_copy(tot[:], tot_ps[:])
        nc.sync.dma_start(out, tot[:])

    nc.compile()
    return nc


class TileCtx:
    """TileContext + ExitStack as one context manager."""

    def __init__(self, nc):
        self.nc = nc
        self.ctx = ExitStack()

    def __enter__(self):
        self.tc = tile.TileContext(self.nc)
        self.tc.__enter__()
        self.ctx.__enter__()
        return self.tc, self.ctx

    def __exit__(self, *exc):
        self.ctx.__exit__(*exc)
        return self.tc.__exit__(*exc)


def make_in_maps(scores, link_targets, candidate_lengths, cluster_ids, lengths):
    in_maps = []
    for b in range(B):
        cid = cluster_ids[b]
        aux = np.empty((P, 4 * T + C), np.float32)
        aux[:, 0 * T : 1 * T] = (
            np.where(cid >= 0, cid, -1).astype(np.float32).reshape(T, P).T
        )
        aux[:, 1 * T : 2 * T] = (
            candidate_lengths[b].astype(np.float32).reshape(T, P).T
        )
        aux[:, 2 * T : 3 * T] = (cid >= 0).astype(np.float32).reshape(T, P).T
        aux[:, 3 * T : 4 * T] = (
            (np.arange(M) < lengths[b]).astype(np.float32).reshape(T, P).T
        )
        aux[:, 4 * T :] = np.arange(C, dtype=np.float32)[None, :]
        in_maps.append(
            {
                "scores": np.ascontiguousarray(scores[b]),
                "linkf": link_targets[b].astype(np.float32),
                "aux": aux,
                "cidb": np.where(cid >= 0, cid, -2).astype(np.float32),
            }
        )
    return in_maps


def kernel(scores, link_targets, candidate_lengths, cluster_ids, lengths):
    if "nc" not in _CACHE:
        _CACHE["nc"] = _build_program()
    nc = _CACHE["nc"]
    in_maps = make_in_maps(
        scores, link_targets, candidate_lengths, cluster_ids, lengths
    )
    res = run_bass_kernel_spmd(nc, in_maps, list(range(B)))
    partials = [res.results[c]["out"].reshape(()) for c in range(B)]
    return np.sum(np.stack(partials), dtype=np.float32)
